# revision 16
# baseline (speedup 1.0000x reference)
"""Trainium2 Bass kernel for nn_Attention_33 (9-tile Restormer-style channel attention).

Work decomposition: 9 tiles x 4 batch = 36 independent items, each [C=128, 128, 128].
8 cores x 5 slots (4 dummy). Per item (all heavy tensors bf16, PSUM f32):

  x --cast--> xb --PE transpose--> xT [n%128, (blk, c)]
  LN stats per token via bn_stats on xT (free-dim reduce), apply per 128-block
  (tensor_scalar, per-partition mu/rstd), PE transpose back -> x~ [c, n].
  ln_w/ln_b are folded into the qkv weights/bias host-side.
  qkv = W'^T @ x~ (PE, bf16), bias folded into dw bias.
  depthwise 3x3: flat layout with 130-el zero guards; 9 taps as fused
  (shift * w_tap + acc) scalar_tensor_tensor on DVE; odd shifts read a
  1-shifted copy (Bch) to keep 4B alignment; row-wrap errors at w=0/127
  fixed by 6 small corrective taps with negated weights.
  L2 norms: ACT Square with accum_out (per-chunk row sums).
  Gram = qT^T @ kT accumulated over 128 token-blocks (PE transposes stream
  per chunk). Norm/temp scaling applied on the tiny [128,128] Gram via two
  PE transposes (row scale each side). Masked softmax (block-diag heads).
  A @ dwconv(v) folded: 9 taps of (A^T * w_v[d,t])^T @ v_shifted accumulate in
  PSUM over a row-padded v layout (130x130, zero borders -> no fixups).
  proj + grw*x residual: two accumulating matmuls per chunk (projT, grw*I).
"""
import numpy as np

B, C, H, W = 4, 128, 384, 384
HEADS = 8
T = 9
HH = WW = 128
N = HH * WW            # 16384 tokens per item
N_CORES = 8
S = 5                  # slots per core
G = 130                # guard elements each side of flat q/k pad
FLATQ = N + 2 * G      # 16644
CH = 2048              # stream chunk (16 rows)
NCH = N // CH          # 8
PC = 1024              # psum chunk (free)
NPC = N // PC          # 16

_cache = {}


def _build_nc(S=S):
    import concourse.bass as bass
    import concourse.tile as tile
    from concourse import mybir
    from concourse.bacc import Bacc
    from concourse.masks import make_identity

    f32 = mybir.dt.float32
    bf16 = mybir.dt.bfloat16
    AX = mybir.AxisListType
    OP = mybir.AluOpType
    AF = mybir.ActivationFunctionType

    nc = Bacc()

    x_in = nc.dram_tensor("x_in", [S, C, N], f32, kind="ExternalInput")
    wqkvT = nc.dram_tensor("wqkvT", [S, C, 3 * C], bf16, kind="ExternalInput")
    dww = nc.dram_tensor("dww", [S, C, 2, 9], f32, kind="ExternalInput")
    dwwE = nc.dram_tensor("dwwE", [S, C, 2, 6], f32, kind="ExternalInput")
    dwv = nc.dram_tensor("dwv", [S, C, 9], f32, kind="ExternalInput")
    dwb = nc.dram_tensor("dwb", [S, C, 3], f32, kind="ExternalInput")
    projTd = nc.dram_tensor("projTd", [S, C, C], bf16, kind="ExternalInput")
    projbd = nc.dram_tensor("projbd", [S, C, 1], f32, kind="ExternalInput")
    grwId = nc.dram_tensor("grwId", [S, C, C], bf16, kind="ExternalInput")
    tempcd = nc.dram_tensor("tempcd", [S, C, 1], f32, kind="ExternalInput")
    maskd = nc.dram_tensor("maskd", [C, C], f32, kind="ExternalInput")
    out_d = nc.dram_tensor("out", [S, C, N], f32, kind="ExternalOutput")

    from contextlib import ExitStack

    with tile.TileContext(nc) as tc, ExitStack() as es:
        consts = es.enter_context(tc.tile_pool(name="consts", bufs=1))
        params = es.enter_context(tc.tile_pool(name="params", bufs=2))
        xs_pool = es.enter_context(tc.tile_pool(name="xs", bufs=2))
        big1 = es.enter_context(tc.tile_pool(name="big1", bufs=1))
        stats_pool = es.enter_context(tc.tile_pool(name="stats", bufs=2))
        pads = es.enter_context(tc.tile_pool(name="pads", bufs=2))
        accs = es.enter_context(tc.tile_pool(name="accs", bufs=3))
        bchs = es.enter_context(tc.tile_pool(name="bchs", bufs=2))
        sqjs = es.enter_context(tc.tile_pool(name="sqjs", bufs=1))
        tbs = es.enter_context(tc.tile_pool(name="tbs", bufs=8))
        smalls = es.enter_context(tc.tile_pool(name="smalls", bufs=2))
        atts = es.enter_context(tc.tile_pool(name="atts", bufs=2))
        outs_pool = es.enter_context(tc.tile_pool(name="outs", bufs=2))
        tpp = es.enter_context(tc.tile_pool(name="tpp", bufs=2, space="PSUM"))
        mmp = es.enter_context(tc.tile_pool(name="mmp", bufs=2, space="PSUM"))
        gsm = es.enter_context(tc.tile_pool(name="gsm", bufs=1, space="PSUM"))

        ident = consts.tile([C, C], bf16)
        make_identity(nc, ident)
        mask_sb = consts.tile([C, C], f32)
        nc.gpsimd.dma_start(out=mask_sb, in_=maskd[:, :])
        eps_ln = consts.tile([C, 1], f32)
        nc.vector.memset(eps_ln, 1e-5)
        eps_l2 = consts.tile([C, 1], f32)
        nc.vector.memset(eps_l2, 1e-24)

        def emit_slot(s):
                # ---- per-slot params ----
                wqkv_sb = params.tile([C, 3 * C], bf16, tag="wqkv")
                nc.gpsimd.dma_start(out=wqkv_sb, in_=wqkvT[s])
                dww_sb = params.tile([C, 2, 9], f32, tag="dww")
                nc.gpsimd.dma_start(out=dww_sb, in_=dww[s])
                dwwE_sb = params.tile([C, 2, 6], f32, tag="dwwE")
                nc.gpsimd.dma_start(out=dwwE_sb, in_=dwwE[s])
                dwv_sb = params.tile([C, 9], f32, tag="dwv")
                nc.gpsimd.dma_start(out=dwv_sb, in_=dwv[s])
                dwb_sb = params.tile([C, 3], f32, tag="dwb")
                nc.gpsimd.dma_start(out=dwb_sb, in_=dwb[s])
                projT_sb = params.tile([C, C], bf16, tag="projT")
                nc.gpsimd.dma_start(out=projT_sb, in_=projTd[s])
                projb_sb = params.tile([C, 1], f32, tag="projb")
                nc.gpsimd.dma_start(out=projb_sb, in_=projbd[s])
                grwI_sb = params.tile([C, C], bf16, tag="grwI")
                nc.gpsimd.dma_start(out=grwI_sb, in_=grwId[s])
                tempc_sb = params.tile([C, 1], f32, tag="tempc")
                nc.gpsimd.dma_start(out=tempc_sb, in_=tempcd[s])

                # ---- load + cast x ----
                xb = big1.tile([C, N], bf16, tag="xb")
                for j in range(N // PC):
                    x32 = xs_pool.tile([C, PC], f32, tag="x32")
                    nc.gpsimd.dma_start(out=x32, in_=x_in[s][:, j * PC:(j + 1) * PC])
                    nc.gpsimd.tensor_copy(out=xb[:, j * PC:(j + 1) * PC], in_=x32)

                # ---- transpose x -> xT  [n%128, (blk, c)] ----
                xT = big1.tile([C, 128, C], bf16, tag="xT")
                for q in range(32):
                    tp = tpp.tile([C, 512], f32, tag="tp")
                    for i in range(4):
                        b = 4 * q + i
                        nc.tensor.matmul(tp[:, i * 128:(i + 1) * 128],
                                         xb[:, b * 128:(b + 1) * 128], ident,
                                         start=True, stop=True)
                    nc.vector.tensor_copy(out=xT[:, 4 * q:4 * q + 4, :], in_=tp)

                # ---- LN stats (per token over c) ----
                st = stats_pool.tile([C, 128, 6], f32, tag="st")
                for b in range(128):
                    nc.vector.bn_stats(out=st[:, b, :], in_=xT[:, b, :])
                mu = stats_pool.tile([C, 128], f32, tag="mu")
                rstd = stats_pool.tile([C, 128], f32, tag="rstd")
                tmp1 = stats_pool.tile([C, 128], f32, tag="tmp1")
                tmp2 = stats_pool.tile([C, 128], f32, tag="tmp2")
                # mean = (Em + Om)/2 ; var = (Ev + Ov)/128 + (Em - Om)^2/4
                Em, Ev = st[:, :, 1], st[:, :, 2]
                Om, Ov = st[:, :, 4], st[:, :, 5]
                nc.vector.tensor_add(out=mu, in0=Em, in1=Om)
                nc.vector.tensor_scalar_mul(out=mu, in0=mu, scalar1=0.5)
                nc.vector.tensor_sub(out=tmp1, in0=Em, in1=Om)
                nc.vector.tensor_mul(out=tmp1, in0=tmp1, in1=tmp1)
                nc.vector.tensor_add(out=tmp2, in0=Ev, in1=Ov)
                nc.vector.tensor_scalar_mul(out=tmp2, in0=tmp2, scalar1=1.0 / 128.0)
                nc.vector.scalar_tensor_tensor(out=rstd, in0=tmp1, scalar=0.25,
                                               in1=tmp2, op0=OP.mult, op1=OP.add)
                # rstd = 1/sqrt(var + eps)
                nc.scalar.activation(out=rstd, in_=rstd, func=AF.Sqrt, bias=eps_ln,
                                     scale=1.0)
                nc.vector.reciprocal(out=rstd, in_=rstd)

                # ---- apply LN per token-block (in place on xT) ----
                for b in range(128):
                    nc.vector.tensor_scalar(out=xT[:, b, :], in0=xT[:, b, :],
                                            scalar1=mu[:, b:b + 1],
                                            scalar2=rstd[:, b:b + 1],
                                            op0=OP.subtract, op1=OP.mult)

                # ---- transpose back -> x~ [c, n] (in place into xT's storage) ----
                xflat = xT[:].rearrange("p a b -> p (a b)")
                for q in range(32):
                    tp = tpp.tile([C, 512], f32, tag="tp")
                    for i in range(4):
                        b = 4 * q + i
                        nc.tensor.matmul(tp[:, i * 128:(i + 1) * 128],
                                         xT[:, b, :], ident, start=True, stop=True)
                    nc.vector.tensor_copy(out=xflat[:, 512 * q:512 * (q + 1)], in_=tp)

                # ---- qkv matmuls for q,k -> flat padded tiles ----
                pad_q = pads.tile([C, FLATQ], bf16, tag="pad")
                pad_k = pads.tile([C, FLATQ], bf16, tag="pad")
                for g, padt in ((0, pad_q), (1, pad_k)):
                    nc.gpsimd.memset(padt[:, 0:G], 0.0)
                    nc.gpsimd.memset(padt[:, G + N:FLATQ], 0.0)
                    for j in range(NPC):
                        pm = mmp.tile([C, PC], f32, tag="mm")
                        for h in range(2):
                            nc.tensor.matmul(
                                pm[:, h * 512:(h + 1) * 512],
                                wqkv_sb[:, g * C:(g + 1) * C],
                                xflat[:, j * PC + h * 512:j * PC + (h + 1) * 512],
                                start=True, stop=True)
                        nc.scalar.activation(out=padt[:, G + j * PC:G + (j + 1) * PC],
                                             in_=pm, func=AF.Copy)

                # ---- depthwise conv (q,k) + stream transposes + Gram ----
                gram = gsm.tile([C, C], f32, tag="gram")
                sqq = smalls.tile([C, NCH], f32, tag="sqq")
                sqk = smalls.tile([C, NCH], f32, tag="sqk")
                # taps: t = kr*3+kc ; delta = (kr-1)*128 + (kc-1)
                deltas = [(t, (t // 3 - 1) * 128 + (t % 3 - 1)) for t in range(9)]
                even_t = [t for t, d in deltas if d % 2 == 0 and t != 4]
                odd_t = [t for t, d in deltas if d % 2 != 0]
                dmap = dict(deltas)

                for ch in range(NCH):
                    base = ch * CH
                    qtb_list, ktb_list = [], []
                    for g, padt, sqc in ((0, pad_q, sqq), (1, pad_k, sqk)):
                        pf = padt[:].rearrange("p a -> p a") if False else padt
                        acc = accs.tile([C, CH], bf16, tag="acc")
                        bch = bchs.tile([C, 2308], bf16, tag="bch")
                        nc.vector.tensor_copy(out=bch[:, 0:2306],
                                              in_=pf[:, base + 1:base + 2307])
                        # center tap + folded bias
                        nc.vector.tensor_scalar(
                            out=acc, in0=pf[:, G + base:G + base + CH],
                            scalar1=dww_sb[:, g, 4:5], scalar2=dwb_sb[:, g:g + 1],
                            op0=OP.mult, op1=OP.add)
                        for t in even_t:
                            d = dmap[t]
                            nc.vector.scalar_tensor_tensor(
                                out=acc, in0=pf[:, G + base + d:G + base + d + CH],
                                scalar=dww_sb[:, g, t:t + 1], in1=acc,
                                op0=OP.mult, op1=OP.add)
                        for t in odd_t:
                            d = dmap[t]
                            nc.vector.scalar_tensor_tensor(
                                out=acc, in0=bch[:, 129 + d:129 + d + CH],
                                scalar=dww_sb[:, g, t:t + 1], in1=acc,
                                op0=OP.mult, op1=OP.add)
                        # edge fixups: remove row-wrap contributions
                        accv = acc[:].rearrange("p (r w) -> p r w", w=128)
                        for idx, dr in enumerate((-1, 0, 1)):
                            off = G + (16 * ch + dr - 1) * 128 + 127
                            src = pf[:, off:off + 15 * 128 + 1:128]
                            nc.vector.scalar_tensor_tensor(
                                out=accv[:, :, 0], in0=src,
                                scalar=dwwE_sb[:, g, idx:idx + 1],
                                in1=accv[:, :, 0], op0=OP.mult, op1=OP.add)
                        for idx, dr in enumerate((-1, 0, 1)):
                            off = G + (16 * ch + dr + 1) * 128
                            src = pf[:, off:off + 15 * 128 + 1:128]
                            nc.vector.scalar_tensor_tensor(
                                out=accv[:, :, 127], in0=src,
                                scalar=dwwE_sb[:, g, 3 + idx:4 + idx],
                                in1=accv[:, :, 127], op0=OP.mult, op1=OP.add)
                        # L2 partial sums
                        sqj = sqjs.tile([C, CH], bf16, tag="sqj")
                        nc.scalar.activation(out=sqj, in_=acc, func=AF.Square,
                                             accum_out=sqc[:, ch:ch + 1])
                        # transpose the 16 blocks of this chunk
                        for q in range(4):
                            tp = tpp.tile([C, 512], f32, tag="tp")
                            for i in range(4):
                                b = 4 * q + i
                                nc.tensor.matmul(tp[:, i * 128:(i + 1) * 128],
                                                 acc[:, b * 128:(b + 1) * 128],
                                                 ident, start=True, stop=True)
                            tb = tbs.tile([C, 512], bf16, tag="tb")
                            nc.vector.tensor_copy(out=tb, in_=tp)
                            (qtb_list if g == 0 else ktb_list).append(tb)
                    # Gram accumulation over this chunk's 16 blocks
                    for q in range(4):
                        for i in range(4):
                            first = ch == 0 and q == 0 and i == 0
                            last = ch == NCH - 1 and q == 3 and i == 3
                            nc.tensor.matmul(gram,
                                             qtb_list[q][:, i * 128:(i + 1) * 128],
                                             ktb_list[q][:, i * 128:(i + 1) * 128],
                                             start=first, stop=last)

                # ---- L2 norms -> row/col scales ----
                rsq_q = smalls.tile([C, 1], f32, tag="rsq_q")
                rsq_k = smalls.tile([C, 1], f32, tag="rsq_k")
                for sqc, rsq in ((sqq, rsq_q), (sqk, rsq_k)):
                    nc.vector.reduce_sum(out=rsq, in_=sqc, axis=AX.X)
                    nc.scalar.activation(out=rsq, in_=rsq, func=AF.Sqrt,
                                         bias=eps_l2, scale=1.0)
                    nc.vector.reciprocal(out=rsq, in_=rsq)
                rqt = smalls.tile([C, 1], f32, tag="rqt")
                nc.vector.tensor_mul(out=rqt, in0=rsq_q, in1=tempc_sb)

                # ---- scale Gram (rows: rqt, cols: rsq_k) + mask + softmax ----
                gs = smalls.tile([C, C], bf16, tag="gs")
                nc.vector.tensor_copy(out=gs, in_=gram)
                gt_ps = gsm.tile([C, C], f32, tag="gsm2")
                nc.tensor.matmul(gt_ps, gs, ident, start=True, stop=True)
                gts = smalls.tile([C, C], bf16, tag="gts")
                nc.vector.tensor_scalar_mul(out=gts, in0=gt_ps, scalar1=rsq_k)
                g2_ps = gsm.tile([C, C], f32, tag="gsm2")
                nc.tensor.matmul(g2_ps, gts, ident, start=True, stop=True)
                lg = smalls.tile([C, C], f32, tag="lg")
                nc.vector.scalar_tensor_tensor(out=lg, in0=g2_ps, scalar=rqt,
                                               in1=mask_sb, op0=OP.mult, op1=OP.add)
                mx = smalls.tile([C, 1], f32, tag="mx")
                nc.vector.reduce_max(out=mx, in_=lg, axis=AX.X)
                nc.vector.tensor_scalar_mul(out=mx, in0=mx, scalar1=-1.0)
                ex = smalls.tile([C, C], f32, tag="ex")
                sume = smalls.tile([C, 1], f32, tag="sume")
                nc.scalar.activation(out=ex, in_=lg, func=AF.Exp, bias=mx,
                                     scale=1.0, accum_out=sume)
                nc.vector.reciprocal(out=sume, in_=sume)
                A_bf = smalls.tile([C, C], bf16, tag="A_bf")
                nc.vector.tensor_scalar_mul(out=A_bf, in0=ex, scalar1=sume)
                at_ps = gsm.tile([C, C], f32, tag="gsm2")
                nc.tensor.matmul(at_ps, A_bf, ident, start=True, stop=True)
                AT_bf = smalls.tile([C, C], bf16, tag="AT_bf")
                nc.vector.tensor_copy(out=AT_bf, in_=at_ps)
                # A_t = AT * w_v[d, t] (per-partition d)
                a_ts = []
                for t in range(9):
                    a_t = smalls.tile([C, C], bf16, tag=f"a_t{t}")
                    nc.vector.tensor_scalar_mul(out=a_t, in0=AT_bf,
                                                scalar1=dwv_sb[:, t:t + 1])
                    a_ts.append(a_t)
                # abv = A @ dwb_v  (bias of v's depthwise, folded through attn)
                dwbv_bf = smalls.tile([C, 1], bf16, tag="dwbv")
                nc.vector.tensor_copy(out=dwbv_bf, in_=dwb_sb[:, 2:3])
                abv_ps = gsm.tile([C, C], f32, tag="gsm2")
                nc.tensor.matmul(abv_ps[:, 0:1], AT_bf, dwbv_bf,
                                 start=True, stop=True)
                abv = smalls.tile([C, 1], f32, tag="abv")
                nc.vector.tensor_copy(out=abv, in_=abv_ps[:, 0:1])

                # ---- v: qkv matmul into row-padded layout [130, 130] ----
                vpad = pads.tile([C, 130, 130], bf16, tag="pad")
                vflat = vpad[:].rearrange("p a b -> p (a b)")
                nc.gpsimd.memset(vflat[:, 0:131], 0.0)
                nc.gpsimd.memset(vflat[:, 129 * 130:130 * 130], 0.0)
                nc.gpsimd.memset(vflat[:, 259:129 * 130:130], 0.0)
                nc.gpsimd.memset(vflat[:, 260:129 * 130:130], 0.0)
                for j in range(NPC):
                    pm = mmp.tile([C, PC], f32, tag="mm")
                    for h in range(2):
                        nc.tensor.matmul(
                            pm[:, h * 512:(h + 1) * 512],
                            wqkv_sb[:, 2 * C:3 * C],
                            xflat[:, j * PC + h * 512:j * PC + (h + 1) * 512],
                            start=True, stop=True)
                    nc.scalar.activation(out=vpad[:, 1 + 8 * j:9 + 8 * j, 1:129],
                                         in_=pm, func=AF.Copy)

                # ---- attn @ dwconv(v) + proj + residual, streamed ----
                for j in range(NPC):
                    pm = mmp.tile([C, PC], f32, tag="mm")
                    for t in range(9):
                        kr, kc = t // 3, t % 3
                        for h in range(2):
                            r0 = 8 * j + 4 * h + kr
                            rhs = vpad[:, r0:r0 + 4, kc:kc + 128]
                            nc.tensor.matmul(pm[:, h * 512:(h + 1) * 512],
                                             a_ts[t], rhs,
                                             start=(t == 0), stop=(t == 8))
                    att = atts.tile([C, PC], bf16, tag="att")
                    nc.scalar.activation(out=att, in_=pm, func=AF.Identity,
                                         bias=abv, scale=1.0)
                    pm2 = mmp.tile([C, PC], f32, tag="mm")
                    for h in range(2):
                        nc.tensor.matmul(pm2[:, h * 512:(h + 1) * 512], projT_sb,
                                         att[:, h * 512:(h + 1) * 512],
                                         start=True, stop=False)
                    for h in range(2):
                        nc.tensor.matmul(pm2[:, h * 512:(h + 1) * 512], grwI_sb,
                                         xb[:, j * PC + h * 512:j * PC + (h + 1) * 512],
                                         start=False, stop=True)
                    oc = outs_pool.tile([C, PC], f32, tag="oc")
                    nc.scalar.activation(out=oc, in_=pm2, func=AF.Identity,
                                         bias=projb_sb, scale=1.0)
                    nc.gpsimd.dma_start(out=out_d[s][:, j * PC:(j + 1) * PC], in_=oc)

        for s in range(S):
            emit_slot(s)

    nc.compile()
    return nc


def _prep_params(inputs):
    """Fold ln affine + qkv bias into weights; build all per-tile param arrays."""
    ln_w, ln_b = inputs["ln_w"], inputs["ln_b"]
    qkv_w, qkv_b = inputs["qkv_w"], inputs["qkv_b"]
    dw_w, dw_b = inputs["dw_w"], inputs["dw_b"]
    proj_w, proj_b = inputs["proj_w"], inputs["proj_b"]
    temp, grw = inputs["temperature"], inputs["grw"]

    P = {}
    eye = np.eye(C, dtype=np.float32)
    for t in range(T):
        Wp = qkv_w[t] * ln_w[t][None, :]                     # [384,128]
        beta = qkv_b[t] + qkv_w[t] @ ln_b[t]                 # [384]
        w9 = dw_w[t].reshape(3 * C, 9).astype(np.float32)    # [384,9]
        dwb_eff = dw_b[t] + beta * w9.sum(-1)                # [384]
        P[t] = dict(
            wqkvT=np.ascontiguousarray(Wp.T),                # [128,384]
            dww=np.stack([w9[0:C], w9[C:2 * C]], axis=1),    # [128,2,9]
            dwwE=-np.stack([w9[0:C][:, [0, 3, 6, 2, 5, 8]],
                            w9[C:2 * C][:, [0, 3, 6, 2, 5, 8]]], axis=1),
            dwv=np.ascontiguousarray(w9[2 * C:3 * C]),       # [128,9]
            dwb=np.ascontiguousarray(dwb_eff.reshape(3, C).T),   # [128,3]
            projT=np.ascontiguousarray(proj_w[t].T),
            projb=proj_b[t][:, None].astype(np.float32),
            grwI=(grw[t] * eye),
            tempc=np.repeat(temp[t], C // HEADS)[:, None].astype(np.float32),
        )
    return P


def _core_maps(inputs):
    x = np.asarray(inputs["x"], np.float32)
    tiles = x.reshape(B, C, 3, HH, 3, WW).transpose(2, 4, 0, 1, 3, 5) \
             .reshape(T, B, C, HH, WW)
    P = _prep_params(inputs)
    mask = np.full((C, C), -1e9, np.float32)
    for h in range(HEADS):
        mask[h * 16:(h + 1) * 16, h * 16:(h + 1) * 16] = 0.0

    bf = np.dtype("bfloat16") if hasattr(np, "bfloat16") else None
    import ml_dtypes
    bf16 = ml_dtypes.bfloat16

    in_maps = []
    for c in range(N_CORES):
        b = c // 2
        tlist = [0, 1, 2, 3, 4] if c % 2 == 0 else [5, 6, 7, 8, 0]
        m = dict(
            x_in=np.stack([tiles[t, b].reshape(C, N) for t in tlist]),
            wqkvT=np.stack([P[t]["wqkvT"] for t in tlist]).astype(bf16),
            dww=np.stack([P[t]["dww"] for t in tlist]).astype(np.float32),
            dwwE=np.stack([P[t]["dwwE"] for t in tlist]).astype(np.float32),
            dwv=np.stack([P[t]["dwv"] for t in tlist]).astype(np.float32),
            dwb=np.stack([P[t]["dwb"] for t in tlist]).astype(np.float32),
            projTd=np.stack([P[t]["projT"] for t in tlist]).astype(bf16),
            projbd=np.stack([P[t]["projb"] for t in tlist]).astype(np.float32),
            grwId=np.stack([P[t]["grwI"] for t in tlist]).astype(bf16),
            tempcd=np.stack([P[t]["tempc"] for t in tlist]).astype(np.float32),
            maskd=mask,
        )
        in_maps.append(m)
    return in_maps


def _get_nc():
    if "nc" not in _cache:
        _cache["nc"] = _build_nc()
    return _cache["nc"]


def _get_runner():
    """Build (once) a jitted shard_map over the 8 axon cores.

    Mirrors concourse.bass2jax.run_bass_via_pjrt's multi-core path but caches
    the jitted callable so repeat calls don't re-trace, and skips output-buffer
    donation (this kernel writes every output element) so device-resident
    input arrays stay valid across timed calls.
    """
    if "runner" in _cache:
        return _cache["runner"]
    import jax
    import concourse.mybir as mybir
    from concourse.bass2jax import (_bass_exec_p, install_neuronx_cc_hook)
    from jax.experimental.shard_map import shard_map
    from jax.sharding import Mesh, PartitionSpec

    nc = _get_nc()
    install_neuronx_cc_hook()
    from concourse.bass2jax import partition_id_tensor
    partition_name = (nc.partition_id_tensor.name
                      if nc.partition_id_tensor else None)
    in_names, out_names, out_avals, zero_outs = [], [], [], []
    for alloc in nc.m.functions[0].allocations:
        if not isinstance(alloc, mybir.MemoryLocationSet):
            continue
        name = alloc.memorylocations[0].name
        if alloc.kind == "ExternalInput":
            if name != partition_name:
                in_names.append(name)
        elif alloc.kind == "ExternalOutput":
            shape = tuple(alloc.tensor_shape)
            dtype = mybir.dt.np(alloc.dtype)
            out_names.append(name)
            out_avals.append(jax.core.ShapedArray(shape, dtype))
            zero_outs.append(np.zeros(shape, dtype))
    n_params = len(in_names)
    all_names = in_names + out_names
    if partition_name is not None:
        all_names = all_names + [partition_name]

    def _body(*args):
        operands = list(args)
        if partition_name is not None:
            operands.append(partition_id_tensor())
        outs = _bass_exec_p.bind(
            *operands,
            out_avals=tuple(out_avals),
            in_names=tuple(all_names),
            out_names=tuple(out_names),
            lowering_input_output_aliases=(),
            sim_require_finite=True,
            sim_require_nnan=True,
            nc=nc,
        )
        return tuple(outs)

    devices = jax.devices()[:N_CORES]
    mesh = Mesh(np.asarray(devices), ("core",))
    nin = n_params + len(out_names)
    sharded = jax.jit(
        shard_map(_body, mesh=mesh,
                  in_specs=(PartitionSpec("core"),) * nin,
                  out_specs=(PartitionSpec("core"),) * len(out_names),
                  check_rep=False),
        keep_unused=True,
    )
    _cache["runner"] = (sharded, in_names, out_names, out_avals, zero_outs, mesh)
    return _cache["runner"]


def _device_args(in_maps):
    """Concatenate per-core inputs along axis 0 (global arrays for shard_map)."""
    sharded, in_names, out_names, out_avals, zero_outs, mesh = _get_runner()
    concat_in = [np.concatenate([m[name] for m in in_maps], axis=0)
                 for name in in_names]
    concat_zero = [np.zeros((N_CORES * z.shape[0], *z.shape[1:]), z.dtype)
                   for z in zero_outs]
    return concat_in + concat_zero


def kernel(x, ln_w, ln_b, qkv_w, qkv_b, dw_w, dw_b, proj_w, proj_b,
           temperature, grw):
    inputs = dict(x=x, ln_w=ln_w, ln_b=ln_b, qkv_w=qkv_w, qkv_b=qkv_b,
                  dw_w=dw_w, dw_b=dw_b, proj_w=proj_w, proj_b=proj_b,
                  temperature=temperature, grw=grw)
    in_maps = _core_maps(inputs)
    sharded, in_names, out_names, out_avals, zero_outs, mesh = _get_runner()
    args = _device_args(in_maps)
    out_arrs = sharded(*args)
    oav = out_avals[0]
    outs = np.asarray(out_arrs[0]).reshape(N_CORES, *oav.shape)

    full = np.empty((T, B, C, HH, WW), np.float32)
    for t in range(T):
        for b in range(B):
            c = 2 * b + (0 if t < 5 else 1)
            s = t if t < 5 else t - 5
            full[t, b] = outs[c][s].reshape(C, HH, WW)
    out = full.reshape(3, 3, B, C, HH, WW).transpose(2, 3, 0, 4, 1, 5) \
              .reshape(B, C, H, W)
    return out.astype(np.float32)


# revision 26
# speedup vs baseline: 15.6563x; 15.6563x over previous
"""Trainium2 Bass kernel for nn_Attention_33 (9-tile Restormer-style channel attention).

Work decomposition: 9 tiles x 4 batch = 36 independent items, each [C=128, 128, 128].
8 cores x 5 slots (4 dummy). Per item (all heavy tensors bf16, PSUM f32):

  x --cast--> xb --PE transpose--> xT [n%128, (blk, c)]
  LN stats per token via bn_stats on xT (free-dim reduce), apply per 128-block
  (tensor_scalar, per-partition mu/rstd), PE transpose back -> x~ [c, n].
  ln_w/ln_b are folded into the qkv weights/bias host-side.
  qkv = W'^T @ x~ (PE, bf16), bias folded into dw bias.
  depthwise 3x3: flat layout with 130-el zero guards; 9 taps as fused
  (shift * w_tap + acc) scalar_tensor_tensor on DVE; odd shifts read a
  1-shifted copy (Bch) to keep 4B alignment; row-wrap errors at w=0/127
  fixed by 6 small corrective taps with negated weights.
  L2 norms: ACT Square with accum_out (per-chunk row sums).
  Gram = qT^T @ kT accumulated over 128 token-blocks (PE transposes stream
  per chunk). Norm/temp scaling applied on the tiny [128,128] Gram via two
  PE transposes (row scale each side). Masked softmax (block-diag heads).
  A @ dwconv(v) folded: 9 taps of (A^T * w_v[d,t])^T @ v_shifted accumulate in
  PSUM over a row-padded v layout (130x130, zero borders -> no fixups).
  proj + grw*x residual: two accumulating matmuls per chunk (projT, grw*I).
"""
import numpy as np

B, C, H, W = 4, 128, 384, 384
HEADS = 8
T = 9
HH = WW = 128
N = HH * WW            # 16384 tokens per item
N_CORES = 8
S = 5                  # slots per core
G = 130                # guard elements each side of flat q/k pad
FLATQ = N + 2 * G      # 16644
CH = 2048              # stream chunk (16 rows)
NCH = N // CH          # 8
PC = 1024              # psum chunk (free)
NPC = N // PC          # 16

_cache = {}


def _build_nc(S=S):
    import concourse.bass as bass
    import concourse.tile as tile
    from concourse import mybir
    from concourse.bacc import Bacc
    from concourse.masks import make_identity

    f32 = mybir.dt.float32
    bf16 = mybir.dt.bfloat16
    AX = mybir.AxisListType
    OP = mybir.AluOpType
    AF = mybir.ActivationFunctionType

    nc = Bacc()

    x_in = nc.dram_tensor("x_in", [S, C, N], f32, kind="ExternalInput")
    wtap = nc.dram_tensor("wtap", [S, 9, C, 2 * C], bf16, kind="ExternalInput")
    wvp = nc.dram_tensor("wvp", [S, C, C], bf16, kind="ExternalInput")
    dwv = nc.dram_tensor("dwv", [S, C, 9], f32, kind="ExternalInput")
    dwb = nc.dram_tensor("dwb", [S, C, 3], f32, kind="ExternalInput")
    projTd = nc.dram_tensor("projTd", [S, C, C], bf16, kind="ExternalInput")
    projbd = nc.dram_tensor("projbd", [S, C, 1], f32, kind="ExternalInput")
    grwId = nc.dram_tensor("grwId", [S, C, C], bf16, kind="ExternalInput")
    tempcd = nc.dram_tensor("tempcd", [S, C, 1], f32, kind="ExternalInput")
    maskd = nc.dram_tensor("maskd", [C, C], f32, kind="ExternalInput")
    out_d = nc.dram_tensor("out", [S, C, N], f32, kind="ExternalOutput")

    from contextlib import ExitStack

    with tile.TileContext(nc) as tc, ExitStack() as es:
        consts = es.enter_context(tc.tile_pool(name="consts", bufs=1))
        params = es.enter_context(tc.tile_pool(name="params", bufs=2))
        xs_pool = es.enter_context(tc.tile_pool(name="xs", bufs=3))
        big1 = es.enter_context(tc.tile_pool(name="big1", bufs=1))
        stats_pool = es.enter_context(tc.tile_pool(name="stats", bufs=1))
        pads = es.enter_context(tc.tile_pool(name="pads", bufs=1))
        accs = es.enter_context(tc.tile_pool(name="accs", bufs=4))
        bchs = es.enter_context(tc.tile_pool(name="bchs", bufs=2))
        sqjs = es.enter_context(tc.tile_pool(name="sqjs", bufs=2))
        tbs = es.enter_context(tc.tile_pool(name="tbs", bufs=8))
        smalls = es.enter_context(tc.tile_pool(name="smalls", bufs=2))
        atts = es.enter_context(tc.tile_pool(name="atts", bufs=3))
        outs_pool = es.enter_context(tc.tile_pool(name="outs", bufs=3))
        tpp = es.enter_context(tc.tile_pool(name="tpp", bufs=2, space="PSUM"))
        mmp = es.enter_context(tc.tile_pool(name="mmp", bufs=2, space="PSUM"))
        gsm = es.enter_context(tc.tile_pool(name="gsm", bufs=1, space="PSUM"))

        ident = consts.tile([C, C], bf16)
        make_identity(nc, ident)
        mask_sb = consts.tile([C, C], f32)
        nc.gpsimd.dma_start(out=mask_sb, in_=maskd[:, :])
        eps_ln = consts.tile([C, 1], f32)
        nc.vector.memset(eps_ln, 1e-5)
        eps_l2 = consts.tile([C, 1], f32)
        nc.vector.memset(eps_l2, 1e-24)

        def emit_slot(s):
                # ---- per-slot params ----
                wtap_sb = params.tile([C, 9, 2 * C], bf16, tag="wtap")
                for t in range(9):
                    nc.gpsimd.dma_start(out=wtap_sb[:, t, :], in_=wtap[s][t])
                wvp_sb = params.tile([C, C], bf16, tag="wvp")
                nc.gpsimd.dma_start(out=wvp_sb, in_=wvp[s])
                dwv_sb = params.tile([C, 9], f32, tag="dwv")
                nc.gpsimd.dma_start(out=dwv_sb, in_=dwv[s])
                dwb_sb = params.tile([C, 3], f32, tag="dwb")
                nc.gpsimd.dma_start(out=dwb_sb, in_=dwb[s])
                projT_sb = params.tile([C, C], bf16, tag="projT")
                nc.gpsimd.dma_start(out=projT_sb, in_=projTd[s])
                projb_sb = params.tile([C, 1], f32, tag="projb")
                nc.gpsimd.dma_start(out=projb_sb, in_=projbd[s])
                grwI_sb = params.tile([C, C], bf16, tag="grwI")
                nc.gpsimd.dma_start(out=grwI_sb, in_=grwId[s])
                tempc_sb = params.tile([C, 1], f32, tag="tempc")
                nc.gpsimd.dma_start(out=tempc_sb, in_=tempcd[s])

                # ---- load + cast x ----
                xb = big1.tile([C, N], bf16, tag="xb", bufs=2)
                for j in range(N // PC):
                    x32 = xs_pool.tile([C, PC], f32, tag="x32")
                    nc.sync.dma_start(out=x32, in_=x_in[s][:, j * PC:(j + 1) * PC])
                    nc.gpsimd.tensor_copy(out=xb[:, j * PC:(j + 1) * PC], in_=x32)

                # ---- transpose x -> xT  [n%128, (blk, c)] ----
                xT = big1.tile([C, 128, C], bf16, tag="xT")
                for q in range(32):
                    tp = tpp.tile([C, 512], f32, tag="tp")
                    for i in range(4):
                        b = 4 * q + i
                        nc.tensor.matmul(tp[:, i * 128:(i + 1) * 128],
                                         xb[:, b * 128:(b + 1) * 128], ident,
                                         start=True, stop=True)
                    nc.vector.tensor_copy(out=xT[:, 4 * q:4 * q + 4, :], in_=tp)

                # ---- LN stats (per token over c) ----
                st = stats_pool.tile([C, 128, 6], f32, tag="st")
                for b in range(128):
                    nc.vector.bn_stats(out=st[:, b, :], in_=xT[:, b, :])
                mu = stats_pool.tile([C, 128], f32, tag="mu")
                rstd = stats_pool.tile([C, 128], f32, tag="rstd")
                tmp1 = stats_pool.tile([C, 128], f32, tag="tmp1")
                tmp2 = stats_pool.tile([C, 128], f32, tag="tmp2")
                # mean = (Em + Om)/2 ; var = (Ev + Ov)/128 + (Em - Om)^2/4
                Em, Ev = st[:, :, 1], st[:, :, 2]
                Om, Ov = st[:, :, 4], st[:, :, 5]
                nc.vector.tensor_add(out=mu, in0=Em, in1=Om)
                nc.vector.tensor_scalar_mul(out=mu, in0=mu, scalar1=0.5)
                nc.vector.tensor_sub(out=tmp1, in0=Em, in1=Om)
                nc.vector.tensor_mul(out=tmp1, in0=tmp1, in1=tmp1)
                nc.vector.tensor_add(out=tmp2, in0=Ev, in1=Ov)
                nc.vector.tensor_scalar_mul(out=tmp2, in0=tmp2, scalar1=1.0 / 128.0)
                nc.vector.scalar_tensor_tensor(out=rstd, in0=tmp1, scalar=0.25,
                                               in1=tmp2, op0=OP.mult, op1=OP.add)
                # rstd = 1/sqrt(var + eps)
                nc.scalar.activation(out=rstd, in_=rstd, func=AF.Sqrt, bias=eps_ln,
                                     scale=1.0)
                nc.vector.reciprocal(out=rstd, in_=rstd)

                # ---- apply LN per token-block (in place on xT) ----
                for b in range(128):
                    nc.vector.tensor_scalar(out=xT[:, b, :], in0=xT[:, b, :],
                                            scalar1=mu[:, b:b + 1],
                                            scalar2=rstd[:, b:b + 1],
                                            op0=OP.subtract, op1=OP.mult)

                # ---- transpose back -> row-padded x~ (xp [130,130], zero borders) ----
                xp = pads.tile([C, 130, 130], bf16, tag="pad")
                xpf = xp[:].rearrange("p a b -> p (a b)")
                nc.gpsimd.memset(xpf[:, 0:131], 0.0)
                nc.gpsimd.memset(xpf[:, 129 * 130:130 * 130], 0.0)
                nc.gpsimd.memset(xpf[:, 259:129 * 130:130], 0.0)
                nc.gpsimd.memset(xpf[:, 260:129 * 130:130], 0.0)
                for q in range(32):
                    tp = tpp.tile([C, 512], f32, tag="tp")
                    for i in range(4):
                        b = 4 * q + i
                        nc.tensor.matmul(tp[:, i * 128:(i + 1) * 128],
                                         xT[:, b, :], ident, start=True, stop=True)
                    nc.vector.tensor_copy(out=xp[:, 1 + 4 * q:5 + 4 * q, 1:129],
                                          in_=tp)

                # ---- fused qkv+depthwise for q,k: 9 tap matmuls per chunk ----
                gram = gsm.tile([C, C], f32, tag="gram")
                sqq = smalls.tile([C, NPC], f32, tag="sqq")
                sqk = smalls.tile([C, NPC], f32, tag="sqk")
                for pc in range(NPC):
                    qtb_list, ktb_list = [], []
                    for g, sqc in ((0, sqq), (1, sqk)):
                        pm = mmp.tile([C, PC], f32, tag="mm")
                        for t in range(9):
                            kr, kc = t // 3, t % 3
                            for h in range(2):
                                r0 = 8 * pc + 4 * h + kr
                                rhs = xp[:, r0:r0 + 4, kc:kc + 128]
                                nc.tensor.matmul(
                                    pm[:, h * 512:(h + 1) * 512],
                                    wtap_sb[:, t, g * C:(g + 1) * C], rhs,
                                    start=(t == 0), stop=(t == 8))
                        acc = accs.tile([C, PC], bf16, tag="acc")
                        nc.vector.tensor_scalar(out=acc, in0=pm,
                                                scalar1=dwb_sb[:, g:g + 1],
                                                scalar2=None, op0=OP.add)
                        sqj = sqjs.tile([C, PC], bf16, tag="sqj")
                        nc.scalar.activation(out=sqj, in_=acc, func=AF.Square,
                                             accum_out=sqc[:, pc:pc + 1])
                        for q in range(2):
                            tp = tpp.tile([C, 512], f32, tag="tp")
                            for i in range(4):
                                b = 4 * q + i
                                nc.tensor.matmul(tp[:, i * 128:(i + 1) * 128],
                                                 acc[:, b * 128:(b + 1) * 128],
                                                 ident, start=True, stop=True)
                            tb = tbs.tile([C, 512], bf16, tag="tb")
                            nc.vector.tensor_copy(out=tb, in_=tp)
                            (qtb_list if g == 0 else ktb_list).append(tb)
                    for q in range(2):
                        for i in range(4):
                            first = pc == 0 and q == 0 and i == 0
                            last = pc == NPC - 1 and q == 1 and i == 3
                            nc.tensor.matmul(gram,
                                             qtb_list[q][:, i * 128:(i + 1) * 128],
                                             ktb_list[q][:, i * 128:(i + 1) * 128],
                                             start=first, stop=last)

                # ---- L2 norms -> row/col scales ----
                rsq_q = smalls.tile([C, 1], f32, tag="rsq_q")
                rsq_k = smalls.tile([C, 1], f32, tag="rsq_k")
                for sqc, rsq in ((sqq, rsq_q), (sqk, rsq_k)):
                    nc.vector.reduce_sum(out=rsq, in_=sqc, axis=AX.X)
                    nc.scalar.activation(out=rsq, in_=rsq, func=AF.Sqrt,
                                         bias=eps_l2, scale=1.0)
                    nc.vector.reciprocal(out=rsq, in_=rsq)
                rqt = smalls.tile([C, 1], f32, tag="rqt")
                nc.vector.tensor_mul(out=rqt, in0=rsq_q, in1=tempc_sb)

                # ---- scale Gram (rows: rqt, cols: rsq_k) + mask + softmax ----
                gs = smalls.tile([C, C], bf16, tag="gs")
                nc.vector.tensor_copy(out=gs, in_=gram)
                gt_ps = gsm.tile([C, C], f32, tag="gsm2")
                nc.tensor.matmul(gt_ps, gs, ident, start=True, stop=True)
                gts = smalls.tile([C, C], bf16, tag="gts")
                nc.vector.tensor_scalar_mul(out=gts, in0=gt_ps, scalar1=rsq_k)
                g2_ps = gsm.tile([C, C], f32, tag="gsm2")
                nc.tensor.matmul(g2_ps, gts, ident, start=True, stop=True)
                lg = smalls.tile([C, C], f32, tag="lg")
                nc.vector.scalar_tensor_tensor(out=lg, in0=g2_ps, scalar=rqt,
                                               in1=mask_sb, op0=OP.mult, op1=OP.add)
                mx = smalls.tile([C, 1], f32, tag="mx")
                nc.vector.reduce_max(out=mx, in_=lg, axis=AX.X)
                nc.vector.tensor_scalar_mul(out=mx, in0=mx, scalar1=-1.0)
                ex = smalls.tile([C, C], f32, tag="ex")
                sume = smalls.tile([C, 1], f32, tag="sume")
                nc.scalar.activation(out=ex, in_=lg, func=AF.Exp, bias=mx,
                                     scale=1.0, accum_out=sume)
                nc.vector.reciprocal(out=sume, in_=sume)
                A_bf = smalls.tile([C, C], bf16, tag="A_bf")
                nc.vector.tensor_scalar_mul(out=A_bf, in0=ex, scalar1=sume)
                at_ps = gsm.tile([C, C], f32, tag="gsm2")
                nc.tensor.matmul(at_ps, A_bf, ident, start=True, stop=True)
                AT_bf = smalls.tile([C, C], bf16, tag="AT_bf")
                nc.vector.tensor_copy(out=AT_bf, in_=at_ps)
                # A_t = AT * w_v[d, t]; M_t = Wv' @ A_t (fold v-projection in)
                a_ts = []
                for t in range(9):
                    a_t = smalls.tile([C, C], bf16, tag=f"a_t{t}", bufs=1)
                    nc.vector.tensor_scalar_mul(out=a_t, in0=AT_bf,
                                                scalar1=dwv_sb[:, t:t + 1])
                    m_ps = gsm.tile([C, C], f32, tag="gsm2")
                    nc.tensor.matmul(m_ps, wvp_sb, a_t, start=True, stop=True)
                    m_t = smalls.tile([C, C], bf16, tag=f"m_t{t}", bufs=1)
                    nc.vector.tensor_copy(out=m_t, in_=m_ps)
                    a_ts.append(m_t)
                # abv = A @ dwb_v  (bias of v's depthwise, folded through attn)
                dwbv_bf = smalls.tile([C, 1], bf16, tag="dwbv")
                nc.vector.tensor_copy(out=dwbv_bf, in_=dwb_sb[:, 2:3])
                abv_ps = gsm.tile([C, C], f32, tag="gsm2")
                nc.tensor.matmul(abv_ps[:, 0:1], AT_bf, dwbv_bf,
                                 start=True, stop=True)
                abv = smalls.tile([C, 1], f32, tag="abv")
                nc.vector.tensor_copy(out=abv, in_=abv_ps[:, 0:1])

                # ---- attn @ dwconv(v) + proj + residual, streamed ----
                for j in range(NPC):
                    pm = mmp.tile([C, PC], f32, tag="mm")
                    for t in range(9):
                        kr, kc = t // 3, t % 3
                        for h in range(2):
                            r0 = 8 * j + 4 * h + kr
                            rhs = xp[:, r0:r0 + 4, kc:kc + 128]
                            nc.tensor.matmul(pm[:, h * 512:(h + 1) * 512],
                                             a_ts[t], rhs,
                                             start=(t == 0), stop=(t == 8))
                    att = atts.tile([C, PC], bf16, tag="att")
                    nc.scalar.activation(out=att, in_=pm, func=AF.Identity,
                                         bias=abv, scale=1.0)
                    pm2 = mmp.tile([C, PC], f32, tag="mm")
                    for h in range(2):
                        nc.tensor.matmul(pm2[:, h * 512:(h + 1) * 512], projT_sb,
                                         att[:, h * 512:(h + 1) * 512],
                                         start=True, stop=False)
                    for h in range(2):
                        nc.tensor.matmul(pm2[:, h * 512:(h + 1) * 512], grwI_sb,
                                         xb[:, j * PC + h * 512:j * PC + (h + 1) * 512],
                                         start=False, stop=True)
                    oc = outs_pool.tile([C, PC], f32, tag="oc")
                    nc.scalar.activation(out=oc, in_=pm2, func=AF.Identity,
                                         bias=projb_sb, scale=1.0)
                    nc.sync.dma_start(out=out_d[s][:, j * PC:(j + 1) * PC], in_=oc)

        for s in range(S):
            emit_slot(s)

    nc.compile()
    return nc


def _prep_params(inputs):
    """Fold ln affine + qkv bias into weights; build all per-tile param arrays."""
    ln_w, ln_b = inputs["ln_w"], inputs["ln_b"]
    qkv_w, qkv_b = inputs["qkv_w"], inputs["qkv_b"]
    dw_w, dw_b = inputs["dw_w"], inputs["dw_b"]
    proj_w, proj_b = inputs["proj_w"], inputs["proj_b"]
    temp, grw = inputs["temperature"], inputs["grw"]

    P = {}
    eye = np.eye(C, dtype=np.float32)
    for t in range(T):
        Wp = qkv_w[t] * ln_w[t][None, :]                     # [384,128]
        beta = qkv_b[t] + qkv_w[t] @ ln_b[t]                 # [384]
        w9 = dw_w[t].reshape(3 * C, 9).astype(np.float32)    # [384,9]
        dwb_eff = dw_b[t] + beta * w9.sum(-1)                # [384]
        wt = np.empty((9, C, 2 * C), np.float32)
        for tap in range(9):
            for g in range(2):
                blk = Wp[g * C:(g + 1) * C] * w9[g * C:(g + 1) * C, tap:tap + 1]
                wt[tap][:, g * C:(g + 1) * C] = blk.T
        P[t] = dict(
            wtap=wt,                                         # [9,128,256]
            wvp=np.ascontiguousarray(Wp[2 * C:3 * C]),       # [128,128] (d, c')
            dwv=np.ascontiguousarray(w9[2 * C:3 * C]),       # [128,9]
            dwb=np.ascontiguousarray(dwb_eff.reshape(3, C).T),   # [128,3]
            projT=np.ascontiguousarray(proj_w[t].T),
            projb=proj_b[t][:, None].astype(np.float32),
            grwI=(grw[t] * eye),
            tempc=np.repeat(temp[t], C // HEADS)[:, None].astype(np.float32),
        )
    return P


def _core_maps(inputs):
    x = np.asarray(inputs["x"], np.float32)
    tiles = x.reshape(B, C, 3, HH, 3, WW).transpose(2, 4, 0, 1, 3, 5) \
             .reshape(T, B, C, HH, WW)
    P = _prep_params(inputs)
    mask = np.full((C, C), -1e9, np.float32)
    for h in range(HEADS):
        mask[h * 16:(h + 1) * 16, h * 16:(h + 1) * 16] = 0.0

    bf = np.dtype("bfloat16") if hasattr(np, "bfloat16") else None
    import ml_dtypes
    bf16 = ml_dtypes.bfloat16

    in_maps = []
    for c in range(N_CORES):
        b = c // 2
        tlist = [0, 1, 2, 3, 4] if c % 2 == 0 else [5, 6, 7, 8, 0]
        m = dict(
            x_in=np.stack([tiles[t, b].reshape(C, N) for t in tlist]),
            wtap=np.stack([P[t]["wtap"] for t in tlist]).astype(bf16),
            wvp=np.stack([P[t]["wvp"] for t in tlist]).astype(bf16),
            dwv=np.stack([P[t]["dwv"] for t in tlist]).astype(np.float32),
            dwb=np.stack([P[t]["dwb"] for t in tlist]).astype(np.float32),
            projTd=np.stack([P[t]["projT"] for t in tlist]).astype(bf16),
            projbd=np.stack([P[t]["projb"] for t in tlist]).astype(np.float32),
            grwId=np.stack([P[t]["grwI"] for t in tlist]).astype(bf16),
            tempcd=np.stack([P[t]["tempc"] for t in tlist]).astype(np.float32),
            maskd=mask,
        )
        in_maps.append(m)
    return in_maps


def _get_nc():
    if "nc" not in _cache:
        _cache["nc"] = _build_nc()
    return _cache["nc"]


def _get_runner():
    """Build (once) a jitted shard_map over the 8 axon cores.

    Mirrors concourse.bass2jax.run_bass_via_pjrt's multi-core path but caches
    the jitted callable so repeat calls don't re-trace, and skips output-buffer
    donation (this kernel writes every output element) so device-resident
    input arrays stay valid across timed calls.
    """
    if "runner" in _cache:
        return _cache["runner"]
    import jax
    import concourse.mybir as mybir
    from concourse.bass2jax import (_bass_exec_p, install_neuronx_cc_hook)
    from jax.experimental.shard_map import shard_map
    from jax.sharding import Mesh, PartitionSpec

    nc = _get_nc()
    install_neuronx_cc_hook()
    from concourse.bass2jax import partition_id_tensor
    partition_name = (nc.partition_id_tensor.name
                      if nc.partition_id_tensor else None)
    in_names, out_names, out_avals, zero_outs = [], [], [], []
    for alloc in nc.m.functions[0].allocations:
        if not isinstance(alloc, mybir.MemoryLocationSet):
            continue
        name = alloc.memorylocations[0].name
        if alloc.kind == "ExternalInput":
            if name != partition_name:
                in_names.append(name)
        elif alloc.kind == "ExternalOutput":
            shape = tuple(alloc.tensor_shape)
            dtype = mybir.dt.np(alloc.dtype)
            out_names.append(name)
            out_avals.append(jax.core.ShapedArray(shape, dtype))
            zero_outs.append(np.zeros(shape, dtype))
    n_params = len(in_names)
    all_names = in_names + out_names
    if partition_name is not None:
        all_names = all_names + [partition_name]

    def _body(*args):
        operands = list(args)
        if partition_name is not None:
            operands.append(partition_id_tensor())
        outs = _bass_exec_p.bind(
            *operands,
            out_avals=tuple(out_avals),
            in_names=tuple(all_names),
            out_names=tuple(out_names),
            lowering_input_output_aliases=(),
            sim_require_finite=True,
            sim_require_nnan=True,
            nc=nc,
        )
        return tuple(outs)

    devices = jax.devices()[:N_CORES]
    mesh = Mesh(np.asarray(devices), ("core",))
    nin = n_params + len(out_names)
    sharded = jax.jit(
        shard_map(_body, mesh=mesh,
                  in_specs=(PartitionSpec("core"),) * nin,
                  out_specs=(PartitionSpec("core"),) * len(out_names),
                  check_rep=False),
        keep_unused=True,
    )
    _cache["runner"] = (sharded, in_names, out_names, out_avals, zero_outs, mesh)
    return _cache["runner"]


def _device_args(in_maps):
    """Concatenate per-core inputs along axis 0 (global arrays for shard_map)."""
    sharded, in_names, out_names, out_avals, zero_outs, mesh = _get_runner()
    concat_in = [np.concatenate([m[name] for m in in_maps], axis=0)
                 for name in in_names]
    concat_zero = [np.zeros((N_CORES * z.shape[0], *z.shape[1:]), z.dtype)
                   for z in zero_outs]
    return concat_in + concat_zero


def kernel(x, ln_w, ln_b, qkv_w, qkv_b, dw_w, dw_b, proj_w, proj_b,
           temperature, grw):
    inputs = dict(x=x, ln_w=ln_w, ln_b=ln_b, qkv_w=qkv_w, qkv_b=qkv_b,
                  dw_w=dw_w, dw_b=dw_b, proj_w=proj_w, proj_b=proj_b,
                  temperature=temperature, grw=grw)
    in_maps = _core_maps(inputs)
    sharded, in_names, out_names, out_avals, zero_outs, mesh = _get_runner()
    args = _device_args(in_maps)
    out_arrs = sharded(*args)
    oav = out_avals[0]
    outs = np.asarray(out_arrs[0]).reshape(N_CORES, *oav.shape)

    full = np.empty((T, B, C, HH, WW), np.float32)
    for t in range(T):
        for b in range(B):
            c = 2 * b + (0 if t < 5 else 1)
            s = t if t < 5 else t - 5
            full[t, b] = outs[c][s].reshape(C, HH, WW)
    out = full.reshape(3, 3, B, C, HH, WW).transpose(2, 3, 0, 4, 1, 5) \
              .reshape(B, C, H, W)
    return out.astype(np.float32)


# revision 32
# speedup vs baseline: 16.1232x; 1.0298x over previous
"""Trainium2 Bass kernel for nn_Attention_33 (9-tile Restormer-style channel attention).

Work decomposition: 9 tiles x 4 batch = 36 independent items, each [C=128, 128, 128].
8 cores x 5 slots (4 dummy). Per item (all heavy tensors bf16, PSUM f32):

  x --cast--> xb --PE transpose--> xT [n%128, (blk, c)]
  LN stats per token via bn_stats on xT (free-dim reduce), apply per 128-block
  (tensor_scalar, per-partition mu/rstd), PE transpose back -> x~ [c, n].
  ln_w/ln_b are folded into the qkv weights/bias host-side.
  qkv = W'^T @ x~ (PE, bf16), bias folded into dw bias.
  depthwise 3x3: flat layout with 130-el zero guards; 9 taps as fused
  (shift * w_tap + acc) scalar_tensor_tensor on DVE; odd shifts read a
  1-shifted copy (Bch) to keep 4B alignment; row-wrap errors at w=0/127
  fixed by 6 small corrective taps with negated weights.
  L2 norms: ACT Square with accum_out (per-chunk row sums).
  Gram = qT^T @ kT accumulated over 128 token-blocks (PE transposes stream
  per chunk). Norm/temp scaling applied on the tiny [128,128] Gram via two
  PE transposes (row scale each side). Masked softmax (block-diag heads).
  A @ dwconv(v) folded: 9 taps of (A^T * w_v[d,t])^T @ v_shifted accumulate in
  PSUM over a row-padded v layout (130x130, zero borders -> no fixups).
  proj + grw*x residual: two accumulating matmuls per chunk (projT, grw*I).
"""
import numpy as np

B, C, H, W = 4, 128, 384, 384
HEADS = 8
T = 9
HH = WW = 128
N = HH * WW            # 16384 tokens per item
N_CORES = 8
S = 5                  # slots per core
G = 130                # guard elements each side of flat q/k pad
FLATQ = N + 2 * G      # 16644
CH = 2048              # stream chunk (16 rows)
NCH = N // CH          # 8
PC = 1024              # psum chunk (free)
NPC = N // PC          # 16

_cache = {}


def _build_nc(S=S):
    import concourse.bass as bass
    import concourse.tile as tile
    from concourse import mybir
    from concourse.bacc import Bacc
    from concourse.masks import make_identity

    f32 = mybir.dt.float32
    bf16 = mybir.dt.bfloat16
    AX = mybir.AxisListType
    OP = mybir.AluOpType
    AF = mybir.ActivationFunctionType

    nc = Bacc()

    x_in = nc.dram_tensor("x_in", [S, C, N], f32, kind="ExternalInput")
    wtap = nc.dram_tensor("wtap", [S, 9, C, 2 * C], bf16, kind="ExternalInput")
    wvp = nc.dram_tensor("wvp", [S, C, C], bf16, kind="ExternalInput")
    dwv = nc.dram_tensor("dwv", [S, C, 9], f32, kind="ExternalInput")
    dwb = nc.dram_tensor("dwb", [S, C, 3], f32, kind="ExternalInput")
    projTd = nc.dram_tensor("projTd", [S, C, C], bf16, kind="ExternalInput")
    projbd = nc.dram_tensor("projbd", [S, C, 1], f32, kind="ExternalInput")
    grwId = nc.dram_tensor("grwId", [S, C, C], bf16, kind="ExternalInput")
    tempcd = nc.dram_tensor("tempcd", [S, C, 1], f32, kind="ExternalInput")
    maskd = nc.dram_tensor("maskd", [C, C], f32, kind="ExternalInput")
    out_d = nc.dram_tensor("out", [S, C, N], f32, kind="ExternalOutput")

    from contextlib import ExitStack

    with tile.TileContext(nc) as tc, ExitStack() as es:
        consts = es.enter_context(tc.tile_pool(name="consts", bufs=1))
        params = es.enter_context(tc.tile_pool(name="params", bufs=2))
        xs_pool = es.enter_context(tc.tile_pool(name="xs", bufs=3))
        big1 = es.enter_context(tc.tile_pool(name="big1", bufs=1))
        stats_pool = es.enter_context(tc.tile_pool(name="stats", bufs=1))
        pads = es.enter_context(tc.tile_pool(name="pads", bufs=1))
        accs = es.enter_context(tc.tile_pool(name="accs", bufs=4))
        bchs = es.enter_context(tc.tile_pool(name="bchs", bufs=2))
        sqjs = es.enter_context(tc.tile_pool(name="sqjs", bufs=2))
        tbs = es.enter_context(tc.tile_pool(name="tbs", bufs=8))
        smalls = es.enter_context(tc.tile_pool(name="smalls", bufs=2))
        atts = es.enter_context(tc.tile_pool(name="atts", bufs=3))
        outs_pool = es.enter_context(tc.tile_pool(name="outs", bufs=3))
        tpp = es.enter_context(tc.tile_pool(name="tpp", bufs=2, space="PSUM"))
        mmp = es.enter_context(tc.tile_pool(name="mmp", bufs=2, space="PSUM"))
        gsm = es.enter_context(tc.tile_pool(name="gsm", bufs=1, space="PSUM"))

        ident = consts.tile([C, C], bf16)
        make_identity(nc, ident)
        mask_sb = consts.tile([C, C], f32)
        nc.gpsimd.dma_start(out=mask_sb, in_=maskd[:, :])
        eps_ln = consts.tile([C, 1], f32)
        nc.vector.memset(eps_ln, 1e-5)
        eps_l2 = consts.tile([C, 1], f32)
        nc.vector.memset(eps_l2, 1e-24)

        def emit_slot(s):
                # ---- per-slot params ----
                wtap_sb = params.tile([C, 9, 2 * C], bf16, tag="wtap")
                for t in range(9):
                    nc.gpsimd.dma_start(out=wtap_sb[:, t, :], in_=wtap[s][t])
                wvp_sb = params.tile([C, C], bf16, tag="wvp")
                nc.gpsimd.dma_start(out=wvp_sb, in_=wvp[s])
                dwv_sb = params.tile([C, 9], f32, tag="dwv")
                nc.gpsimd.dma_start(out=dwv_sb, in_=dwv[s])
                dwb_sb = params.tile([C, 3], f32, tag="dwb")
                nc.gpsimd.dma_start(out=dwb_sb, in_=dwb[s])
                projT_sb = params.tile([C, C], bf16, tag="projT")
                nc.gpsimd.dma_start(out=projT_sb, in_=projTd[s])
                projb_sb = params.tile([C, 1], f32, tag="projb")
                nc.gpsimd.dma_start(out=projb_sb, in_=projbd[s])
                grwI_sb = params.tile([C, C], bf16, tag="grwI")
                nc.gpsimd.dma_start(out=grwI_sb, in_=grwId[s])
                tempc_sb = params.tile([C, 1], f32, tag="tempc")
                nc.gpsimd.dma_start(out=tempc_sb, in_=tempcd[s])

                # ---- load + cast x ----
                xb = big1.tile([C, N], bf16, tag="xb", bufs=2)
                for j in range(N // PC):
                    x32 = xs_pool.tile([C, PC], f32, tag="x32")
                    nc.sync.dma_start(out=x32, in_=x_in[s][:, j * PC:(j + 1) * PC])
                    nc.gpsimd.tensor_copy(out=xb[:, j * PC:(j + 1) * PC], in_=x32)

                # ---- transpose x -> xT  [n%128, (blk, c)] ----
                xT = big1.tile([C, 128, C], bf16, tag="xT")
                for q in range(32):
                    tp = tpp.tile([C, 512], f32, tag="tp")
                    for i in range(4):
                        b = 4 * q + i
                        nc.tensor.matmul(tp[:, i * 128:(i + 1) * 128],
                                         xb[:, b * 128:(b + 1) * 128], ident,
                                         start=True, stop=True)
                    nc.vector.tensor_copy(out=xT[:, 4 * q:4 * q + 4, :], in_=tp)

                # ---- LN stats (per token over c) ----
                st = stats_pool.tile([C, 128, 6], f32, tag="st")
                for b in range(128):
                    nc.vector.bn_stats(out=st[:, b, :], in_=xT[:, b, :])
                mu = stats_pool.tile([C, 128], f32, tag="mu")
                rstd = stats_pool.tile([C, 128], f32, tag="rstd")
                tmp1 = stats_pool.tile([C, 128], f32, tag="tmp1")
                tmp2 = stats_pool.tile([C, 128], f32, tag="tmp2")
                # mean = (Em + Om)/2 ; var = (Ev + Ov)/128 + (Em - Om)^2/4
                Em, Ev = st[:, :, 1], st[:, :, 2]
                Om, Ov = st[:, :, 4], st[:, :, 5]
                nc.vector.tensor_add(out=mu, in0=Em, in1=Om)
                nc.vector.tensor_scalar_mul(out=mu, in0=mu, scalar1=0.5)
                nc.vector.tensor_sub(out=tmp1, in0=Em, in1=Om)
                nc.vector.tensor_mul(out=tmp1, in0=tmp1, in1=tmp1)
                nc.vector.tensor_add(out=tmp2, in0=Ev, in1=Ov)
                nc.vector.tensor_scalar_mul(out=tmp2, in0=tmp2, scalar1=1.0 / 128.0)
                nc.vector.scalar_tensor_tensor(out=rstd, in0=tmp1, scalar=0.25,
                                               in1=tmp2, op0=OP.mult, op1=OP.add)
                # rstd = 1/sqrt(var + eps)
                nc.scalar.activation(out=rstd, in_=rstd, func=AF.Sqrt, bias=eps_ln,
                                     scale=1.0)
                nc.vector.reciprocal(out=rstd, in_=rstd)

                # ---- apply LN per token-block (in place on xT) ----
                for b in range(128):
                    nc.vector.tensor_scalar(out=xT[:, b, :], in0=xT[:, b, :],
                                            scalar1=mu[:, b:b + 1],
                                            scalar2=rstd[:, b:b + 1],
                                            op0=OP.subtract, op1=OP.mult)

                # ---- transpose back -> row-padded x~ (xp [130,130], zero borders) ----
                xp = pads.tile([C, 130, 130], bf16, tag="pad")
                xpf = xp[:].rearrange("p a b -> p (a b)")
                nc.gpsimd.memset(xpf[:, 0:131], 0.0)
                nc.gpsimd.memset(xpf[:, 129 * 130:130 * 130], 0.0)
                nc.gpsimd.memset(xpf[:, 259:129 * 130:130], 0.0)
                nc.gpsimd.memset(xpf[:, 260:129 * 130:130], 0.0)
                for q in range(32):
                    tp = tpp.tile([C, 512], f32, tag="tp")
                    for i in range(4):
                        b = 4 * q + i
                        nc.tensor.matmul(tp[:, i * 128:(i + 1) * 128],
                                         xT[:, b, :], ident, start=True, stop=True)
                    nc.vector.tensor_copy(out=xp[:, 1 + 4 * q:5 + 4 * q, 1:129],
                                          in_=tp)

                # ---- fused qkv+depthwise for q,k: 9 tap matmuls per chunk ----
                gram = gsm.tile([C, C], f32, tag="gram")
                sqq = smalls.tile([C, NPC], f32, tag="sqq")
                sqk = smalls.tile([C, NPC], f32, tag="sqk")
                for pc in range(NPC):
                    qtb_list, ktb_list = [], []
                    for g, sqc in ((0, sqq), (1, sqk)):
                        pm = mmp.tile([C, PC], f32, tag="mm")
                        for t in range(9):
                            kr, kc = t // 3, t % 3
                            for h in range(2):
                                r0 = 8 * pc + 4 * h + kr
                                rhs = xp[:, r0:r0 + 4, kc:kc + 128]
                                nc.tensor.matmul(
                                    pm[:, h * 512:(h + 1) * 512],
                                    wtap_sb[:, t, g * C:(g + 1) * C], rhs,
                                    start=(t == 0), stop=(t == 8))
                        acc = accs.tile([C, PC], bf16, tag="acc")
                        nc.scalar.activation(out=acc, in_=pm, func=AF.Identity,
                                             bias=dwb_sb[:, g:g + 1], scale=1.0)
                        sqj = sqjs.tile([C, PC], bf16, tag="sqj")
                        nc.scalar.activation(out=sqj, in_=acc, func=AF.Square,
                                             accum_out=sqc[:, pc:pc + 1])
                        for q in range(2):
                            tp = tpp.tile([C, 512], f32, tag="tp")
                            for i in range(4):
                                b = 4 * q + i
                                nc.tensor.matmul(tp[:, i * 128:(i + 1) * 128],
                                                 acc[:, b * 128:(b + 1) * 128],
                                                 ident, start=True, stop=True)
                            tb = tbs.tile([C, 512], bf16, tag="tb")
                            nc.vector.tensor_copy(out=tb, in_=tp)
                            (qtb_list if g == 0 else ktb_list).append(tb)
                    for q in range(2):
                        for i in range(4):
                            first = pc == 0 and q == 0 and i == 0
                            last = pc == NPC - 1 and q == 1 and i == 3
                            nc.tensor.matmul(gram,
                                             qtb_list[q][:, i * 128:(i + 1) * 128],
                                             ktb_list[q][:, i * 128:(i + 1) * 128],
                                             start=first, stop=last)

                # ---- L2 norms -> row/col scales ----
                rsq_q = smalls.tile([C, 1], f32, tag="rsq_q")
                rsq_k = smalls.tile([C, 1], f32, tag="rsq_k")
                for sqc, rsq in ((sqq, rsq_q), (sqk, rsq_k)):
                    nc.vector.reduce_sum(out=rsq, in_=sqc, axis=AX.X)
                    nc.scalar.activation(out=rsq, in_=rsq, func=AF.Sqrt,
                                         bias=eps_l2, scale=1.0)
                    nc.vector.reciprocal(out=rsq, in_=rsq)
                rqt = smalls.tile([C, 1], f32, tag="rqt")
                nc.vector.tensor_mul(out=rqt, in0=rsq_q, in1=tempc_sb)

                # ---- scale Gram (rows: rqt, cols: rsq_k) + mask + softmax ----
                gs = smalls.tile([C, C], bf16, tag="gs")
                nc.vector.tensor_copy(out=gs, in_=gram)
                gt_ps = gsm.tile([C, C], f32, tag="gsm2")
                nc.tensor.matmul(gt_ps, gs, ident, start=True, stop=True)
                gts = smalls.tile([C, C], bf16, tag="gts")
                nc.vector.tensor_scalar_mul(out=gts, in0=gt_ps, scalar1=rsq_k)
                g2_ps = gsm.tile([C, C], f32, tag="gsm2")
                nc.tensor.matmul(g2_ps, gts, ident, start=True, stop=True)
                lg = smalls.tile([C, C], f32, tag="lg")
                nc.vector.scalar_tensor_tensor(out=lg, in0=g2_ps, scalar=rqt,
                                               in1=mask_sb, op0=OP.mult, op1=OP.add)
                mx = smalls.tile([C, 1], f32, tag="mx")
                nc.vector.reduce_max(out=mx, in_=lg, axis=AX.X)
                nc.vector.tensor_scalar_mul(out=mx, in0=mx, scalar1=-1.0)
                ex = smalls.tile([C, C], f32, tag="ex")
                sume = smalls.tile([C, 1], f32, tag="sume")
                nc.scalar.activation(out=ex, in_=lg, func=AF.Exp, bias=mx,
                                     scale=1.0, accum_out=sume)
                nc.vector.reciprocal(out=sume, in_=sume)
                A_bf = smalls.tile([C, C], bf16, tag="A_bf")
                nc.vector.tensor_scalar_mul(out=A_bf, in0=ex, scalar1=sume)
                at_ps = gsm.tile([C, C], f32, tag="gsm2")
                nc.tensor.matmul(at_ps, A_bf, ident, start=True, stop=True)
                AT_bf = smalls.tile([C, C], bf16, tag="AT_bf")
                nc.vector.tensor_copy(out=AT_bf, in_=at_ps)
                # A_t = AT * w_v[d, t]; M_t = Wv' @ A_t (fold v-projection in)
                a_ts = []
                for t in range(9):
                    a_t = smalls.tile([C, C], bf16, tag=f"a_t{t}", bufs=1)
                    nc.vector.tensor_scalar_mul(out=a_t, in0=AT_bf,
                                                scalar1=dwv_sb[:, t:t + 1])
                    m_ps = gsm.tile([C, C], f32, tag="gsm2")
                    nc.tensor.matmul(m_ps, wvp_sb, a_t, start=True, stop=True)
                    m_t = smalls.tile([C, C], bf16, tag=f"m_t{t}", bufs=1)
                    nc.vector.tensor_copy(out=m_t, in_=m_ps)
                    a_ts.append(m_t)
                # abv = A @ dwb_v  (bias of v's depthwise, folded through attn)
                dwbv_bf = smalls.tile([C, 1], bf16, tag="dwbv")
                nc.vector.tensor_copy(out=dwbv_bf, in_=dwb_sb[:, 2:3])
                abv_ps = gsm.tile([C, C], f32, tag="gsm2")
                nc.tensor.matmul(abv_ps[:, 0:1], AT_bf, dwbv_bf,
                                 start=True, stop=True)
                abv = smalls.tile([C, 1], f32, tag="abv")
                nc.vector.tensor_copy(out=abv, in_=abv_ps[:, 0:1])

                # ---- attn @ dwconv(v) + proj + residual, streamed ----
                for j in range(NPC):
                    pm = mmp.tile([C, PC], f32, tag="mm")
                    for t in range(9):
                        kr, kc = t // 3, t % 3
                        for h in range(2):
                            r0 = 8 * j + 4 * h + kr
                            rhs = xp[:, r0:r0 + 4, kc:kc + 128]
                            nc.tensor.matmul(pm[:, h * 512:(h + 1) * 512],
                                             a_ts[t], rhs,
                                             start=(t == 0), stop=(t == 8))
                    att = atts.tile([C, PC], bf16, tag="att")
                    nc.scalar.activation(out=att, in_=pm, func=AF.Identity,
                                         bias=abv, scale=1.0)
                    pm2 = mmp.tile([C, PC], f32, tag="mm")
                    for h in range(2):
                        nc.tensor.matmul(pm2[:, h * 512:(h + 1) * 512], projT_sb,
                                         att[:, h * 512:(h + 1) * 512],
                                         start=True, stop=False)
                    for h in range(2):
                        nc.tensor.matmul(pm2[:, h * 512:(h + 1) * 512], grwI_sb,
                                         xb[:, j * PC + h * 512:j * PC + (h + 1) * 512],
                                         start=False, stop=True)
                    oc = outs_pool.tile([C, PC], f32, tag="oc")
                    nc.scalar.activation(out=oc, in_=pm2, func=AF.Identity,
                                         bias=projb_sb, scale=1.0)
                    nc.sync.dma_start(out=out_d[s][:, j * PC:(j + 1) * PC], in_=oc)

        for s in range(S):
            emit_slot(s)

    nc.compile()
    return nc


def _prep_params(inputs):
    """Fold ln affine + qkv bias into weights; build all per-tile param arrays."""
    ln_w, ln_b = inputs["ln_w"], inputs["ln_b"]
    qkv_w, qkv_b = inputs["qkv_w"], inputs["qkv_b"]
    dw_w, dw_b = inputs["dw_w"], inputs["dw_b"]
    proj_w, proj_b = inputs["proj_w"], inputs["proj_b"]
    temp, grw = inputs["temperature"], inputs["grw"]

    P = {}
    eye = np.eye(C, dtype=np.float32)
    for t in range(T):
        Wp = qkv_w[t] * ln_w[t][None, :]                     # [384,128]
        beta = qkv_b[t] + qkv_w[t] @ ln_b[t]                 # [384]
        w9 = dw_w[t].reshape(3 * C, 9).astype(np.float32)    # [384,9]
        dwb_eff = dw_b[t] + beta * w9.sum(-1)                # [384]
        wt = np.empty((9, C, 2 * C), np.float32)
        for tap in range(9):
            for g in range(2):
                blk = Wp[g * C:(g + 1) * C] * w9[g * C:(g + 1) * C, tap:tap + 1]
                wt[tap][:, g * C:(g + 1) * C] = blk.T
        P[t] = dict(
            wtap=wt,                                         # [9,128,256]
            wvp=np.ascontiguousarray(Wp[2 * C:3 * C]),       # [128,128] (d, c')
            dwv=np.ascontiguousarray(w9[2 * C:3 * C]),       # [128,9]
            dwb=np.ascontiguousarray(dwb_eff.reshape(3, C).T),   # [128,3]
            projT=np.ascontiguousarray(proj_w[t].T),
            projb=proj_b[t][:, None].astype(np.float32),
            grwI=(grw[t] * eye),
            tempc=np.repeat(temp[t], C // HEADS)[:, None].astype(np.float32),
        )
    return P


def _core_maps(inputs):
    x = np.asarray(inputs["x"], np.float32)
    tiles = x.reshape(B, C, 3, HH, 3, WW).transpose(2, 4, 0, 1, 3, 5) \
             .reshape(T, B, C, HH, WW)
    P = _prep_params(inputs)
    mask = np.full((C, C), -1e9, np.float32)
    for h in range(HEADS):
        mask[h * 16:(h + 1) * 16, h * 16:(h + 1) * 16] = 0.0

    bf = np.dtype("bfloat16") if hasattr(np, "bfloat16") else None
    import ml_dtypes
    bf16 = ml_dtypes.bfloat16

    in_maps = []
    for c in range(N_CORES):
        b = c // 2
        tlist = [0, 1, 2, 3, 4] if c % 2 == 0 else [5, 6, 7, 8, 0]
        m = dict(
            x_in=np.stack([tiles[t, b].reshape(C, N) for t in tlist]),
            wtap=np.stack([P[t]["wtap"] for t in tlist]).astype(bf16),
            wvp=np.stack([P[t]["wvp"] for t in tlist]).astype(bf16),
            dwv=np.stack([P[t]["dwv"] for t in tlist]).astype(np.float32),
            dwb=np.stack([P[t]["dwb"] for t in tlist]).astype(np.float32),
            projTd=np.stack([P[t]["projT"] for t in tlist]).astype(bf16),
            projbd=np.stack([P[t]["projb"] for t in tlist]).astype(np.float32),
            grwId=np.stack([P[t]["grwI"] for t in tlist]).astype(bf16),
            tempcd=np.stack([P[t]["tempc"] for t in tlist]).astype(np.float32),
            maskd=mask,
        )
        in_maps.append(m)
    return in_maps


def _get_nc():
    if "nc" not in _cache:
        _cache["nc"] = _build_nc()
    return _cache["nc"]


def _get_runner():
    """Build (once) a jitted shard_map over the 8 axon cores.

    Mirrors concourse.bass2jax.run_bass_via_pjrt's multi-core path but caches
    the jitted callable so repeat calls don't re-trace, and skips output-buffer
    donation (this kernel writes every output element) so device-resident
    input arrays stay valid across timed calls.
    """
    if "runner" in _cache:
        return _cache["runner"]
    import jax
    import concourse.mybir as mybir
    from concourse.bass2jax import (_bass_exec_p, install_neuronx_cc_hook)
    from jax.experimental.shard_map import shard_map
    from jax.sharding import Mesh, PartitionSpec

    nc = _get_nc()
    install_neuronx_cc_hook()
    from concourse.bass2jax import partition_id_tensor
    partition_name = (nc.partition_id_tensor.name
                      if nc.partition_id_tensor else None)
    in_names, out_names, out_avals, zero_outs = [], [], [], []
    for alloc in nc.m.functions[0].allocations:
        if not isinstance(alloc, mybir.MemoryLocationSet):
            continue
        name = alloc.memorylocations[0].name
        if alloc.kind == "ExternalInput":
            if name != partition_name:
                in_names.append(name)
        elif alloc.kind == "ExternalOutput":
            shape = tuple(alloc.tensor_shape)
            dtype = mybir.dt.np(alloc.dtype)
            out_names.append(name)
            out_avals.append(jax.core.ShapedArray(shape, dtype))
            zero_outs.append(np.zeros(shape, dtype))
    n_params = len(in_names)
    all_names = in_names + out_names
    if partition_name is not None:
        all_names = all_names + [partition_name]

    def _body(*args):
        operands = list(args)
        if partition_name is not None:
            operands.append(partition_id_tensor())
        outs = _bass_exec_p.bind(
            *operands,
            out_avals=tuple(out_avals),
            in_names=tuple(all_names),
            out_names=tuple(out_names),
            lowering_input_output_aliases=(),
            sim_require_finite=True,
            sim_require_nnan=True,
            nc=nc,
        )
        return tuple(outs)

    devices = jax.devices()[:N_CORES]
    mesh = Mesh(np.asarray(devices), ("core",))
    nin = n_params + len(out_names)
    sharded = jax.jit(
        shard_map(_body, mesh=mesh,
                  in_specs=(PartitionSpec("core"),) * nin,
                  out_specs=(PartitionSpec("core"),) * len(out_names),
                  check_rep=False),
        keep_unused=True,
    )
    _cache["runner"] = (sharded, in_names, out_names, out_avals, zero_outs, mesh)
    return _cache["runner"]


def _device_args(in_maps):
    """Concatenate per-core inputs along axis 0 (global arrays for shard_map)."""
    sharded, in_names, out_names, out_avals, zero_outs, mesh = _get_runner()
    concat_in = [np.concatenate([m[name] for m in in_maps], axis=0)
                 for name in in_names]
    concat_zero = [np.zeros((N_CORES * z.shape[0], *z.shape[1:]), z.dtype)
                   for z in zero_outs]
    return concat_in + concat_zero


def kernel(x, ln_w, ln_b, qkv_w, qkv_b, dw_w, dw_b, proj_w, proj_b,
           temperature, grw):
    inputs = dict(x=x, ln_w=ln_w, ln_b=ln_b, qkv_w=qkv_w, qkv_b=qkv_b,
                  dw_w=dw_w, dw_b=dw_b, proj_w=proj_w, proj_b=proj_b,
                  temperature=temperature, grw=grw)
    in_maps = _core_maps(inputs)
    sharded, in_names, out_names, out_avals, zero_outs, mesh = _get_runner()
    args = _device_args(in_maps)
    out_arrs = sharded(*args)
    oav = out_avals[0]
    outs = np.asarray(out_arrs[0]).reshape(N_CORES, *oav.shape)

    full = np.empty((T, B, C, HH, WW), np.float32)
    for t in range(T):
        for b in range(B):
            c = 2 * b + (0 if t < 5 else 1)
            s = t if t < 5 else t - 5
            full[t, b] = outs[c][s].reshape(C, HH, WW)
    out = full.reshape(3, 3, B, C, HH, WW).transpose(2, 3, 0, 4, 1, 5) \
              .reshape(B, C, H, W)
    return out.astype(np.float32)
